# revision 3
# baseline (speedup 1.0000x reference)
"""Grouped linear (MoE expert GEMM) on 8 NeuronCores, expert-parallel.

Problem: hidden_states [16384, 2048] f32, weight [8, 2048, 2048] f32,
tokens_per_expert [8] = 2048 each (balanced). Output [16384, 2048] f32 with
out[g*2048+t, o] = sum_i x[g*2048+t, i] * weight[g, o, i].

Sharding: expert-parallel — core g gets expert g's weight [2048, 2048] and its
2048 routed tokens; each core runs one 2048x2048x2048 GEMM. No collectives.

Per-core kernel: inputs are rounded to bf16 on the host (rel err ~2e-3 on the
output, vs the 2e-2 gate). bf16 streams the PE at 1 row/cycle with fast weight
load, giving the 216 ns/matmul floor (512 cols @ 2.4 GHz + NX issue). X^T and
W^T are fully resident in SBUF. Startup is trigger-rate-limited (~0.65 us per
DMA instruction), so inputs go on one HWDGE queue (sync) in consumption order
with W chunked coarsely, outputs on the other (scalar). A short burst of
scratch matmuls warms the PE clock (HAM un-throttle) while the first inputs
stream in.
"""

import numpy as np
import ml_dtypes

G = 8
TPG = 2048  # tokens per expert (= per core)
IN = 2048
OUT = 2048
P = 128
KM = IN // P  # 16 contraction chunks of 128
TT = TPG // P  # 16 token tiles of 128
ON = 4  # number of output-column chunks
OW = OUT // ON  # 512
NWARM = 10  # scratch matmuls to warm the PE clock during input DMA

_nc_cache = {}


def _build_nc():
    import concourse.bacc as bacc
    import concourse.mybir as mybir
    import concourse.tile as tile

    if "nc" in _nc_cache:
        return _nc_cache["nc"]

    f32 = mybir.dt.float32
    bf16 = mybir.dt.bfloat16

    nc = bacc.Bacc(None, target_bir_lowering=False)

    # xt[p, tt, km, tl] = x_core[tt*128+tl, km*128+p]   (X^T, k on partitions)
    xt = nc.dram_tensor("xt", [P, TT, KM, P], bf16, kind="ExternalInput")
    # wt[p, km, oi, o] = w_core[oi*512+o, km*128+p]     (W^T, k on partitions)
    wt = nc.dram_tensor("wt", [P, KM, ON, OW], bf16, kind="ExternalInput")
    # out[tt, p, o] = C[tt*128+p, o]
    out = nc.dram_tensor("out", [TT, P, OUT], f32, kind="ExternalOutput")

    with tile.TileContext(nc) as tc:
        with (
            tc.tile_pool(name="xpool", bufs=1) as xpool,
            tc.tile_pool(name="wpool", bufs=1) as wpool,
            tc.tile_pool(name="opool", bufs=8) as opool,
            tc.tile_pool(name="ppool", bufs=8, space="PSUM") as ppool,
        ):
            # --- PE warmup: scratch matmuls while inputs stream in --------
            scratch = xpool.tile([P, OW], bf16, name="scratch", tag="scratch")
            nc.vector.memset(scratch[:], 0.0)
            wpsum = ppool.tile([P, OW], f32, name="psum", tag="psum")
            for i in range(NWARM):
                nc.tensor.matmul(
                    out=wpsum[:],
                    lhsT=scratch[:, 0:P],
                    rhs=scratch[:],
                    start=(i == 0),
                    stop=(i == NWARM - 1),
                )

            xtiles = [
                xpool.tile([P, KM, P], bf16, name=f"x_sb{tt}", tag=f"x{tt}")
                for tt in range(TT)
            ]
            # W for output chunk 0: four 4-km chunks (fine-grained critical
            # path); output chunks 1..3: one tile each.
            wc0 = [
                wpool.tile([P, 4, OW], bf16, name=f"w_c{c}", tag=f"wc{c}")
                for c in range(4)
            ]
            wrest = [
                wpool.tile([P, KM, OW], bf16, name=f"w_o{oi}", tag=f"wo{oi}")
                for oi in range(1, ON)
            ]

            # Input DMAs on the sync HWDGE queue, in consumption order.
            def in_dma(dst, src):
                nc.sync.dma_start(out=dst, in_=src)

            in_dma(xtiles[0][:], xt[:, 0])
            in_dma(wc0[0][:], wt[:, 0:4, 0])
            in_dma(xtiles[1][:], xt[:, 1])
            in_dma(wc0[1][:], wt[:, 4:8, 0])
            in_dma(wc0[2][:], wt[:, 8:12, 0])
            in_dma(wc0[3][:], wt[:, 12:16, 0])
            in_dma(xtiles[2][:], xt[:, 2])
            in_dma(xtiles[3][:], xt[:, 3])
            in_dma(wrest[0][:], wt[:, :, 1])
            for tt in range(4, 8):
                in_dma(xtiles[tt][:], xt[:, tt])
            in_dma(wrest[1][:], wt[:, :, 2])
            for tt in range(8, 12):
                in_dma(xtiles[tt][:], xt[:, tt])
            in_dma(wrest[2][:], wt[:, :, 3])
            for tt in range(12, TT):
                in_dma(xtiles[tt][:], xt[:, tt])

            def w_slice(km, oi):
                if oi == 0:
                    return wc0[km // 4][:, km % 4, :]
                return wrest[oi - 1][:, km, :]

            for oi in range(ON):
                for tt in range(TT):
                    psum = ppool.tile([P, OW], f32, name="psum", tag="psum")
                    for km in range(KM):
                        nc.tensor.matmul(
                            out=psum[:],
                            lhsT=xtiles[tt][:, km, :],
                            rhs=w_slice(km, oi),
                            start=(km == 0),
                            stop=(km == KM - 1),
                        )
                    o_sb = opool.tile([P, OW], f32, name="o_sb", tag="o_sb")
                    nc.vector.tensor_copy(out=o_sb[:], in_=psum[:])
                    nc.scalar.dma_start(
                        out=out[tt, :, oi * OW : (oi + 1) * OW], in_=o_sb[:]
                    )

    nc.compile()
    _nc_cache["nc"] = nc
    return nc


def _shard_inputs(hidden_states, weight):
    """Host-side reshuffle + bf16 rounding into the kernel's DRAM layouts."""
    bf16 = ml_dtypes.bfloat16
    x = np.asarray(hidden_states, dtype=np.float32).astype(bf16)
    w = np.asarray(weight, dtype=np.float32).astype(bf16)
    in_maps = []
    for g in range(G):
        xg = x[g * TPG : (g + 1) * TPG]  # [2048, 2048]
        # [tt, tl, km, p] -> [p, tt, km, tl]
        xtg = np.ascontiguousarray(xg.reshape(TT, P, KM, P).transpose(3, 0, 2, 1))
        wg = w[g]  # [out, in]
        # [oi, o, km, p] -> [p, km, oi, o]
        wtg = np.ascontiguousarray(wg.reshape(ON, OW, KM, P).transpose(3, 2, 0, 1))
        in_maps.append({"xt": xtg, "wt": wtg})
    return in_maps


def _run(hidden_states, weight, trace=False, tmpdir=None):
    from concourse.bass_utils import run_bass_kernel_spmd

    nc = _build_nc()
    in_maps = _shard_inputs(hidden_states, weight)
    res = run_bass_kernel_spmd(
        nc, in_maps, core_ids=list(range(G)), trace=trace, tmpdir=tmpdir
    )
    outs = [
        np.asarray(res.results[g]["out"]).reshape(TPG, OUT) for g in range(G)
    ]
    full = np.concatenate(outs, axis=0)
    return full, res


def kernel(hidden_states, weight, tokens_per_expert=None, **_ignored):
    out, _ = _run(hidden_states, weight, trace=False)
    return out
